# revision 10
# baseline (speedup 1.0000x reference)
"""Focal contrastive loss on 8 Trainium2 NeuronCores.

Strategy (data-parallel over rows, per-core column permutation):
  - Rows are sorted by label (16 classes). Core r owns 1024 consecutive
    sorted rows, which span 2-3 classes.  Each core receives its own
    column permutation of the full feature set in which the union of the
    classes touching its rows (the "window", <= W cols) comes first.
  - Device pass 1: sim block [1024, 8192] = statT.T @ movT via PE (bf16),
    exp(sim/T) row-sums via ACT accum (totals), masked class sums via DVE.
  - Device pass 2 (window cols only): focal terms
        t = -(1-pt)^2 * log(pt),  pt = sigmoid(z),  z = 10*sim - ln(d)
    computed as (LP - z) * exp(-2*LP) with LP = ln(1 + e^z); only Exp/Ln
    activation functions are used (one ACT table set).
  - Host: weights per-row results by 1/(n_c - 1), reduces in f64.

The device program is identical across cores (SPMD); all per-core
variation lives in the input data (permuted features + masks).
"""

import numpy as np
import ml_dtypes

TEMPERATURE = 0.1
INV_T = 1.0 / TEMPERATURE  # 10.0
EPS = 1e-12

B = 8192
D = 512
M = 8  # cores
R = B // M  # rows per core
KT = D // 128  # contraction tiles
NT = R // 128  # row tiles per core

_cache = {}


def _build(W, nw_chunks):
    """Build the SPMD Bass program for window width W (multiple of 512)."""
    from contextlib import ExitStack
    import concourse.bass as bass
    import concourse.tile as tile
    from concourse import bacc, mybir

    F32 = mybir.dt.float32
    BF16 = mybir.dt.bfloat16
    EXP = mybir.ActivationFunctionType.Exp
    LN = mybir.ActivationFunctionType.Ln
    ADD = mybir.AluOpType.add
    MUL = mybir.AluOpType.mult
    X = mybir.AxisListType.X

    nc = bacc.Bacc("TRN2", target_bir_lowering=False, debug=False)
    mov_d = nc.dram_tensor("mov", [D, B], BF16, kind="ExternalInput").ap()
    stat_d = nc.dram_tensor("stat", [D, R], BF16, kind="ExternalInput").ap()
    mcls_d = nc.dram_tensor("mcls", [R, W], BF16, kind="ExternalInput").ap()
    mpos_d = nc.dram_tensor("mpos", [R, W], BF16, kind="ExternalInput").ap()
    rowsum_d = nc.dram_tensor("rowsum", [R], F32, kind="ExternalOutput").ap()
    neg_d = nc.dram_tensor("neg_exp", [R], F32, kind="ExternalOutput").ap()

    nch = len(nw_chunks)

    with tile.TileContext(nc) as tc, ExitStack() as ctx:
        const = ctx.enter_context(tc.tile_pool(name="const", bufs=1))
        masks = ctx.enter_context(tc.tile_pool(name="masks", bufs=2))
        e1wp = ctx.enter_context(tc.tile_pool(name="e1wp", bufs=2))
        e1nwp = ctx.enter_context(tc.tile_pool(name="e1nwp", bufs=2))
        pw = ctx.enter_context(tc.tile_pool(name="pw", bufs=6))
        small = ctx.enter_context(tc.tile_pool(name="small", bufs=4))
        outp = ctx.enter_context(tc.tile_pool(name="outp", bufs=1))
        psw_pool = ctx.enter_context(tc.tile_pool(name="psw", bufs=1, space="PSUM"))
        psnw_pool = ctx.enter_context(tc.tile_pool(name="psnw", bufs=2, space="PSUM"))

        mov_sb = []
        stat_sb = []
        for k in range(KT):
            t = const.tile([128, B], BF16, tag=f"mov{k}")
            nc.sync.dma_start(out=t, in_=mov_d[128 * k : 128 * (k + 1), :])
            mov_sb.append(t)
            s = const.tile([128, R], BF16, tag=f"stat{k}")
            nc.sync.dma_start(out=s, in_=stat_d[128 * k : 128 * (k + 1), :])
            stat_sb.append(s)

        rowsum_sb = outp.tile([128, NT], F32, tag="rowsum")
        neg_sb = outp.tile([128, NT], F32, tag="negdbg")
        eps_t = const.tile([128, 1], F32, tag="eps")
        nc.vector.memset(eps_t, EPS)

        for i in range(NT):
            mcls_t = masks.tile([128, W], BF16, tag="mcls")
            nc.gpsimd.dma_start(out=mcls_t, in_=mcls_d[128 * i : 128 * (i + 1), :])
            mpos_t = masks.tile([128, W], BF16, tag="mpos")
            nc.gpsimd.dma_start(out=mpos_t, in_=mpos_d[128 * i : 128 * (i + 1), :])

            strip = small.tile([128, 1 + nch], F32, tag="strip")
            stat_i = [stat_sb[k][:, 128 * i : 128 * (i + 1)] for k in range(KT)]

            # --- pass 1, non-window chunks ---
            col = W
            for j, csz in enumerate(nw_chunks):
                ps = psnw_pool.tile([128, csz], F32, tag="nw")
                for c0 in range(0, csz, 512):
                    for k in range(KT):
                        nc.tensor.matmul(
                            ps[:, c0 : c0 + 512],
                            stat_i[k],
                            mov_sb[k][:, col + c0 : col + c0 + 512],
                            start=(k == 0),
                            stop=(k == KT - 1),
                        )
                e1 = e1nwp.tile([128, csz], BF16, tag="e1nw")
                nc.scalar.activation(
                    out=e1, in_=ps, func=EXP, scale=INV_T,
                    accum_out=strip[:, 1 + j : 2 + j],
                )
                col += csz

            # --- pass 1, window chunk (kept in PSUM through pass 2) ---
            psw = psw_pool.tile([128, W], F32, tag="win")
            for c0 in range(0, W, 512):
                for k in range(KT):
                    nc.tensor.matmul(
                        psw[:, c0 : c0 + 512],
                        stat_i[k],
                        mov_sb[k][:, c0 : c0 + 512],
                        start=(k == 0),
                        stop=(k == KT - 1),
                    )
            e1w = e1wp.tile([128, W], F32, tag="e1w")
            nc.scalar.activation(
                out=e1w, in_=psw, func=EXP, scale=INV_T, accum_out=strip[:, 0:1]
            )

            # --- per-row scalars ---
            em = pw.tile([128, W], F32, tag="pw")
            cls = small.tile([128, 1], F32, tag="cls")
            nc.vector.tensor_tensor(out=em, in0=e1w, in1=mcls_t, op=MUL)
            nc.vector.tensor_reduce(out=cls, in_=em, axis=X, op=ADD)
            tot = small.tile([128, 1], F32, tag="tot")
            nc.vector.tensor_reduce(out=tot, in_=strip, axis=X, op=ADD)
            neg = small.tile([128, 1], F32, tag="neg")
            nc.vector.tensor_sub(neg, tot, cls)
            nc.vector.tensor_copy(out=neg_sb[:, i : i + 1], in_=neg)
            L = small.tile([128, 1], F32, tag="L")
            nc.scalar.activation(out=L, in_=neg, func=LN, bias=eps_t, scale=1.0)
            se = small.tile([128, 1], F32, tag="se")
            nc.scalar.activation(out=se, in_=L, func=EXP, scale=-1.0)

            # --- pass 2 (window only) ---
            lp = pw.tile([128, W], F32, tag="pw")
            nc.scalar.activation(out=lp, in_=em, func=LN, scale=se, bias=1.0)
            wt = pw.tile([128, W], F32, tag="pw")
            nc.scalar.activation(out=wt, in_=lp, func=EXP, scale=-2.0)
            t1 = pw.tile([128, W], F32, tag="pw")
            nc.vector.scalar_tensor_tensor(
                out=t1, in0=psw, scalar=-INV_T, in1=lp, op0=MUL, op1=ADD
            )
            t3 = pw.tile([128, W], F32, tag="pw")
            nc.vector.scalar_tensor_tensor(
                out=t3, in0=t1, scalar=L, in1=wt, op0=ADD, op1=MUL
            )
            junk = pw.tile([128, W], F32, tag="pw")
            nc.vector.tensor_tensor(out=junk, in0=t3, in1=mpos_t, op=MUL)
            nc.vector.tensor_reduce(
                out=rowsum_sb[:, i : i + 1], in_=junk, axis=X, op=ADD
            )

        nc.sync.dma_start(
            out=rowsum_d.rearrange("(t p) -> p t", p=128), in_=rowsum_sb
        )
        nc.sync.dma_start(out=neg_d.rearrange("(t p) -> p t", p=128), in_=neg_sb)
    nc.compile()
    return nc


def _eval_class_order(perm_c, counts):
    """Max per-core column-union for a given class ordering."""
    csum = np.concatenate([[0], np.cumsum(counts[perm_c])])
    maxU = 0
    for r in range(M):
        lo_row, hi_row = r * R, (r + 1) * R
        first = int(np.searchsorted(csum, lo_row, side="right")) - 1
        last = int(np.searchsorted(csum, hi_row - 1, side="right")) - 1
        maxU = max(maxU, int(csum[last + 1] - csum[first]))
    return maxU


def _best_class_order(counts):
    """Choose a class ordering that minimizes the max per-core union."""
    ncls = len(counts)
    best = np.arange(ncls)
    bestU = _eval_class_order(best, counts)
    # heuristic: pair largest with smallest
    o = np.argsort(counts)[::-1]
    paired = np.empty(ncls, dtype=np.int64)
    paired[0::2] = o[: ncls // 2]
    paired[1::2] = o[ncls // 2 :][::-1]
    u = _eval_class_order(paired, counts)
    if u < bestU:
        best, bestU = paired, u
    rng = np.random.default_rng(0)
    cand = np.arange(ncls)
    for _ in range(4000):
        rng.shuffle(cand)
        u = _eval_class_order(cand, counts)
        if u < bestU:
            best, bestU = cand.copy(), u
            if bestU <= 1600:
                break
    return best, bestU


def _prep_inputs(features, labels):
    """Host-side sharding: per-core permutations, masks, class weights."""
    labels = np.asarray(labels).astype(np.int64)
    feats = np.asarray(features, dtype=np.float32)
    ncls = int(labels.max()) + 1
    counts = np.bincount(labels, minlength=ncls)
    class_order, maxU = _best_class_order(counts)
    # rank of each class in the chosen ordering
    rank = np.empty(ncls, dtype=np.int64)
    rank[class_order] = np.arange(ncls)
    order = np.argsort(rank[labels], kind="stable")
    sorted_ranks = rank[labels][order]
    bounds_by_rank = np.concatenate(
        [[0], np.cumsum(counts[class_order])]
    )

    cores = []
    for r in range(M):
        rows = order[r * R : (r + 1) * R]
        rks = sorted_ranks[r * R : (r + 1) * R]
        lo = int(bounds_by_rank[rks.min()])
        hi = int(bounds_by_rank[rks.max() + 1])
        cores.append((rows, lo, hi))

    W = max(2048, int(-(-maxU // 512) * 512))
    if W > 3072:
        raise ValueError(f"class window {maxU} too large for PSUM budget")
    nw_total = B - W
    nwc = 1024 if W <= 2048 else 512
    nw_chunks = [nwc] * (nw_total // nwc)
    if nw_total % nwc:
        nw_chunks.append(nw_total % nwc)
    assert sum(nw_chunks) + W == B

    feats_bf = feats.astype(ml_dtypes.bfloat16)
    in_maps = []
    for r in range(M):
        rows, lo, hi = cores[r]
        win = order[lo:hi]
        rest = np.concatenate([order[:lo], order[hi:]])
        pad = W - len(win)
        # pad window with other-class cols (mask kills them)
        perm = np.concatenate([win, rest[:pad], rest[pad:]])
        wlab = labels[perm[:W]]
        rlab = labels[rows]
        eq = (wlab[None, :] == rlab[:, None])
        mcls = eq.astype(ml_dtypes.bfloat16)
        # positive mask: same class, excluding self
        self_col = np.full(R, -1, dtype=np.int64)
        colpos = {int(c): j for j, c in enumerate(perm[:W])}
        for p, g in enumerate(rows):
            self_col[p] = colpos[int(g)]
        mpos = eq.copy()
        mpos[np.arange(R), self_col] = False
        mpos = mpos.astype(ml_dtypes.bfloat16)
        in_maps.append(
            {
                "mov": np.ascontiguousarray(feats_bf[perm].T),
                "stat": np.ascontiguousarray(feats_bf[rows].T),
                "mcls": np.ascontiguousarray(mcls),
                "mpos": np.ascontiguousarray(mpos),
            }
        )
    return W, nw_chunks, in_maps, cores, counts


def _get_program(W, nw_chunks):
    key = (W, tuple(nw_chunks))
    if key not in _cache:
        _cache[key] = _build(W, nw_chunks)
    return _cache[key]


def _run(nc, in_maps, trace=False, trace_kwargs=None):
    from concourse.bass_utils import run_bass_kernel_spmd

    return run_bass_kernel_spmd(
        nc, in_maps, list(range(M)), trace=trace, trace_kwargs=trace_kwargs or {}
    )


def _finish(res_list, cores, counts, labels):
    labels = np.asarray(labels).astype(np.int64)
    w = np.zeros(16, dtype=np.float64)
    for c in range(16):
        n = counts[c]
        if n >= 2 and (B - n) > 0:
            w[c] = 1.0 / (n - 1)
    total = 0.0
    for r in range(M):
        rows, _, _ = cores[r]
        rs = np.asarray(res_list[r]["rowsum"], dtype=np.float64)
        total += float(np.dot(rs, w[labels[rows]]))
    return np.array(total / B, dtype=np.float32)


def kernel(features, labels):
    W, nw_chunks, in_maps, cores, counts = _prep_inputs(features, labels)
    nc = _get_program(W, nw_chunks)
    res = _run(nc, in_maps)
    return _finish(res.results, cores, counts, labels)


# revision 11
# speedup vs baseline: 1.2302x; 1.2302x over previous
"""Focal contrastive loss on 8 Trainium2 NeuronCores.

Strategy (data-parallel over rows, per-core column permutation):
  - Rows are sorted by label (16 classes). Core r owns 1024 consecutive
    sorted rows, which span 2-3 classes.  Each core receives its own
    column permutation of the full feature set in which the union of the
    classes touching its rows (the "window", <= W cols) comes first.
  - Device pass 1: sim block [1024, 8192] = statT.T @ movT via PE (bf16),
    exp(sim/T) row-sums via ACT accum (totals), masked class sums via DVE.
  - Device pass 2 (window cols only): focal terms
        t = -(1-pt)^2 * log(pt),  pt = sigmoid(z),  z = 10*sim - ln(d)
    computed as (LP - z) * exp(-2*LP) with LP = ln(1 + e^z); only Exp/Ln
    activation functions are used (one ACT table set).
  - Host: weights per-row results by 1/(n_c - 1), reduces in f64.

The device program is identical across cores (SPMD); all per-core
variation lives in the input data (permuted features + masks).
"""

import numpy as np
import ml_dtypes

TEMPERATURE = 0.1
INV_T = 1.0 / TEMPERATURE  # 10.0
EPS = 1e-12

B = 8192
D = 512
M = 8  # cores
R = B // M  # rows per core
KT = D // 128  # contraction tiles
NT = R // 128  # row tiles per core

_cache = {}


def _build(W, nw_chunks):
    """Build the SPMD Bass program for window width W (multiple of 512)."""
    from contextlib import ExitStack
    import concourse.bass as bass
    import concourse.tile as tile
    from concourse import bacc, mybir

    F32 = mybir.dt.float32
    BF16 = mybir.dt.bfloat16
    EXP = mybir.ActivationFunctionType.Exp
    LN = mybir.ActivationFunctionType.Ln
    ADD = mybir.AluOpType.add
    MUL = mybir.AluOpType.mult
    X = mybir.AxisListType.X

    nc = bacc.Bacc("TRN2", target_bir_lowering=False, debug=False)
    mov_d = nc.dram_tensor("mov", [D, B], BF16, kind="ExternalInput").ap()
    stat_d = nc.dram_tensor("stat", [D, R], BF16, kind="ExternalInput").ap()
    mcls_d = nc.dram_tensor("mcls", [R, W], BF16, kind="ExternalInput").ap()
    mpos_d = nc.dram_tensor("mpos", [R, W], BF16, kind="ExternalInput").ap()
    rowsum_d = nc.dram_tensor("rowsum", [R], F32, kind="ExternalOutput").ap()
    neg_d = nc.dram_tensor("neg_exp", [R], F32, kind="ExternalOutput").ap()

    nch = len(nw_chunks)

    with tile.TileContext(nc) as tc, ExitStack() as ctx:
        const = ctx.enter_context(tc.tile_pool(name="const", bufs=1))
        masks = ctx.enter_context(tc.tile_pool(name="masks", bufs=2))
        e1wp = ctx.enter_context(tc.tile_pool(name="e1wp", bufs=2))
        e1nwp = ctx.enter_context(tc.tile_pool(name="e1nwp", bufs=2))
        pw = ctx.enter_context(tc.tile_pool(name="pw", bufs=6))
        small = ctx.enter_context(tc.tile_pool(name="small", bufs=4))
        outp = ctx.enter_context(tc.tile_pool(name="outp", bufs=1))
        psw_pool = ctx.enter_context(tc.tile_pool(name="psw", bufs=1, space="PSUM"))
        psnw_pool = ctx.enter_context(tc.tile_pool(name="psnw", bufs=2, space="PSUM"))

        mov_sb = []
        stat_sb = []
        for k in range(KT):
            t = const.tile([128, B], BF16, tag=f"mov{k}")
            nc.sync.dma_start(out=t, in_=mov_d[128 * k : 128 * (k + 1), :])
            mov_sb.append(t)
            s = const.tile([128, R], BF16, tag=f"stat{k}")
            nc.sync.dma_start(out=s, in_=stat_d[128 * k : 128 * (k + 1), :])
            stat_sb.append(s)

        rowsum_sb = outp.tile([128, NT], F32, tag="rowsum")
        neg_sb = outp.tile([128, NT], F32, tag="negdbg")
        eps_t = const.tile([128, 1], F32, tag="eps")
        nc.vector.memset(eps_t, EPS)

        for i in range(NT):
            mcls_t = masks.tile([128, W], BF16, tag="mcls")
            nc.gpsimd.dma_start(out=mcls_t, in_=mcls_d[128 * i : 128 * (i + 1), :])
            mpos_t = masks.tile([128, W], BF16, tag="mpos")
            nc.gpsimd.dma_start(out=mpos_t, in_=mpos_d[128 * i : 128 * (i + 1), :])

            strip = small.tile([128, 1 + nch], F32, tag="strip")
            stat_i = [stat_sb[k][:, 128 * i : 128 * (i + 1)] for k in range(KT)]

            # --- pass 1, non-window chunks ---
            col = W
            for j, csz in enumerate(nw_chunks):
                ps = psnw_pool.tile([128, csz], F32, tag="nw")
                for c0 in range(0, csz, 512):
                    for k in range(KT):
                        nc.tensor.matmul(
                            ps[:, c0 : c0 + 512],
                            stat_i[k],
                            mov_sb[k][:, col + c0 : col + c0 + 512],
                            start=(k == 0),
                            stop=(k == KT - 1),
                        )
                e1 = e1nwp.tile([128, csz], BF16, tag="e1nw")
                nc.scalar.activation(
                    out=e1, in_=ps, func=EXP, scale=INV_T,
                    accum_out=strip[:, 1 + j : 2 + j],
                )
                col += csz

            # --- pass 1, window chunk (kept in PSUM through pass 2) ---
            psw = psw_pool.tile([128, W], F32, tag="win")
            for c0 in range(0, W, 512):
                for k in range(KT):
                    nc.tensor.matmul(
                        psw[:, c0 : c0 + 512],
                        stat_i[k],
                        mov_sb[k][:, c0 : c0 + 512],
                        start=(k == 0),
                        stop=(k == KT - 1),
                    )
            e1w = e1wp.tile([128, W], F32, tag="e1w")
            nc.scalar.activation(
                out=e1w, in_=psw, func=EXP, scale=INV_T, accum_out=strip[:, 0:1]
            )

            # --- per-row scalars ---
            em = pw.tile([128, W], F32, tag="pw")
            cls = small.tile([128, 1], F32, tag="cls")
            nc.vector.tensor_tensor(out=em, in0=e1w, in1=mcls_t, op=MUL)
            nc.vector.tensor_reduce(out=cls, in_=em, axis=X, op=ADD)
            tot = small.tile([128, 1], F32, tag="tot")
            nc.vector.tensor_reduce(out=tot, in_=strip, axis=X, op=ADD)
            neg = small.tile([128, 1], F32, tag="neg")
            nc.vector.tensor_sub(neg, tot, cls)
            nc.vector.tensor_copy(out=neg_sb[:, i : i + 1], in_=neg)
            L = small.tile([128, 1], F32, tag="L")
            nc.scalar.activation(out=L, in_=neg, func=LN, bias=eps_t, scale=1.0)
            se = small.tile([128, 1], F32, tag="se")
            nc.scalar.activation(out=se, in_=L, func=EXP, scale=-1.0)

            # --- pass 2 (window only) ---
            lp = pw.tile([128, W], F32, tag="pw")
            nc.scalar.activation(out=lp, in_=em, func=LN, scale=se, bias=1.0)
            wt = pw.tile([128, W], F32, tag="pw")
            nc.scalar.activation(out=wt, in_=lp, func=EXP, scale=-2.0)
            t1 = pw.tile([128, W], F32, tag="pw")
            nc.vector.scalar_tensor_tensor(
                out=t1, in0=psw, scalar=-INV_T, in1=lp, op0=MUL, op1=ADD
            )
            t3 = pw.tile([128, W], F32, tag="pw")
            nc.vector.scalar_tensor_tensor(
                out=t3, in0=t1, scalar=L, in1=wt, op0=ADD, op1=MUL
            )
            junk = pw.tile([128, W], F32, tag="pw")
            nc.vector.tensor_tensor(out=junk, in0=t3, in1=mpos_t, op=MUL)
            nc.vector.tensor_reduce(
                out=rowsum_sb[:, i : i + 1], in_=junk, axis=X, op=ADD
            )

        nc.sync.dma_start(
            out=rowsum_d.rearrange("(t p) -> p t", p=128), in_=rowsum_sb
        )
        nc.sync.dma_start(out=neg_d.rearrange("(t p) -> p t", p=128), in_=neg_sb)

    # Exp and Ln live in different default ACT table sets; left alone the
    # table-load pass thrashes between them (~33 loads x 1.3us). Restrict
    # selection to the combined natural_log_exp_and_others set (positions
    # preserved so act_func_set_id still indexes act_info.json correctly).
    import concourse.bacc as bacc_mod

    orig_gat = bacc_mod.get_activation_tables

    def gat_combined(arch):
        t = orig_gat(arch)
        return {
            name: (funcs if name == "natural_log_exp_and_others" else set())
            for name, funcs in t.items()
        }

    bacc_mod.get_activation_tables = gat_combined
    try:
        nc.compile()
    finally:
        bacc_mod.get_activation_tables = orig_gat
    return nc


def _eval_class_order(perm_c, counts):
    """Max per-core column-union for a given class ordering."""
    csum = np.concatenate([[0], np.cumsum(counts[perm_c])])
    maxU = 0
    for r in range(M):
        lo_row, hi_row = r * R, (r + 1) * R
        first = int(np.searchsorted(csum, lo_row, side="right")) - 1
        last = int(np.searchsorted(csum, hi_row - 1, side="right")) - 1
        maxU = max(maxU, int(csum[last + 1] - csum[first]))
    return maxU


def _best_class_order(counts):
    """Choose a class ordering that minimizes the max per-core union."""
    ncls = len(counts)
    best = np.arange(ncls)
    bestU = _eval_class_order(best, counts)
    # heuristic: pair largest with smallest
    o = np.argsort(counts)[::-1]
    paired = np.empty(ncls, dtype=np.int64)
    paired[0::2] = o[: ncls // 2]
    paired[1::2] = o[ncls // 2 :][::-1]
    u = _eval_class_order(paired, counts)
    if u < bestU:
        best, bestU = paired, u
    rng = np.random.default_rng(0)
    cand = np.arange(ncls)
    for _ in range(4000):
        rng.shuffle(cand)
        u = _eval_class_order(cand, counts)
        if u < bestU:
            best, bestU = cand.copy(), u
            if bestU <= 1600:
                break
    return best, bestU


def _prep_inputs(features, labels):
    """Host-side sharding: per-core permutations, masks, class weights."""
    labels = np.asarray(labels).astype(np.int64)
    feats = np.asarray(features, dtype=np.float32)
    ncls = int(labels.max()) + 1
    counts = np.bincount(labels, minlength=ncls)
    class_order, maxU = _best_class_order(counts)
    # rank of each class in the chosen ordering
    rank = np.empty(ncls, dtype=np.int64)
    rank[class_order] = np.arange(ncls)
    order = np.argsort(rank[labels], kind="stable")
    sorted_ranks = rank[labels][order]
    bounds_by_rank = np.concatenate(
        [[0], np.cumsum(counts[class_order])]
    )

    cores = []
    for r in range(M):
        rows = order[r * R : (r + 1) * R]
        rks = sorted_ranks[r * R : (r + 1) * R]
        lo = int(bounds_by_rank[rks.min()])
        hi = int(bounds_by_rank[rks.max() + 1])
        cores.append((rows, lo, hi))

    W = max(2048, int(-(-maxU // 512) * 512))
    if W > 3072:
        raise ValueError(f"class window {maxU} too large for PSUM budget")
    nw_total = B - W
    nwc = 1024 if W <= 2048 else 512
    nw_chunks = [nwc] * (nw_total // nwc)
    if nw_total % nwc:
        nw_chunks.append(nw_total % nwc)
    assert sum(nw_chunks) + W == B

    feats_bf = feats.astype(ml_dtypes.bfloat16)
    in_maps = []
    for r in range(M):
        rows, lo, hi = cores[r]
        win = order[lo:hi]
        rest = np.concatenate([order[:lo], order[hi:]])
        pad = W - len(win)
        # pad window with other-class cols (mask kills them)
        perm = np.concatenate([win, rest[:pad], rest[pad:]])
        wlab = labels[perm[:W]]
        rlab = labels[rows]
        eq = (wlab[None, :] == rlab[:, None])
        mcls = eq.astype(ml_dtypes.bfloat16)
        # positive mask: same class, excluding self
        self_col = np.full(R, -1, dtype=np.int64)
        colpos = {int(c): j for j, c in enumerate(perm[:W])}
        for p, g in enumerate(rows):
            self_col[p] = colpos[int(g)]
        mpos = eq.copy()
        mpos[np.arange(R), self_col] = False
        mpos = mpos.astype(ml_dtypes.bfloat16)
        in_maps.append(
            {
                "mov": np.ascontiguousarray(feats_bf[perm].T),
                "stat": np.ascontiguousarray(feats_bf[rows].T),
                "mcls": np.ascontiguousarray(mcls),
                "mpos": np.ascontiguousarray(mpos),
            }
        )
    return W, nw_chunks, in_maps, cores, counts


def _get_program(W, nw_chunks):
    key = (W, tuple(nw_chunks))
    if key not in _cache:
        _cache[key] = _build(W, nw_chunks)
    return _cache[key]


def _run(nc, in_maps, trace=False, trace_kwargs=None):
    from concourse.bass_utils import run_bass_kernel_spmd

    return run_bass_kernel_spmd(
        nc, in_maps, list(range(M)), trace=trace, trace_kwargs=trace_kwargs or {}
    )


def _finish(res_list, cores, counts, labels):
    labels = np.asarray(labels).astype(np.int64)
    w = np.zeros(16, dtype=np.float64)
    for c in range(16):
        n = counts[c]
        if n >= 2 and (B - n) > 0:
            w[c] = 1.0 / (n - 1)
    total = 0.0
    for r in range(M):
        rows, _, _ = cores[r]
        rs = np.asarray(res_list[r]["rowsum"], dtype=np.float64)
        total += float(np.dot(rs, w[labels[rows]]))
    return np.array(total / B, dtype=np.float32)


def kernel(features, labels):
    W, nw_chunks, in_maps, cores, counts = _prep_inputs(features, labels)
    nc = _get_program(W, nw_chunks)
    res = _run(nc, in_maps)
    return _finish(res.results, cores, counts, labels)


# revision 15
# speedup vs baseline: 1.3770x; 1.1193x over previous
"""Focal contrastive loss on 8 Trainium2 NeuronCores.

Strategy (data-parallel over rows, per-core column permutation):
  - Rows are sorted by label (16 classes). Core r owns 1024 consecutive
    sorted rows, which span 2-3 classes.  Each core receives its own
    column permutation of the full feature set in which the union of the
    classes touching its rows (the "window", <= W cols) comes first.
  - Device pass 1: sim block [1024, 8192] = statT.T @ movT via PE (bf16),
    exp(sim/T) row-sums via ACT accum (totals), masked class sums via DVE.
  - Device pass 2 (window cols only): focal terms
        t = -(1-pt)^2 * log(pt),  pt = sigmoid(z),  z = 10*sim - ln(d)
    computed as (LP - z) * exp(-2*LP) with LP = ln(1 + e^z); only Exp/Ln
    activation functions are used (one ACT table set).
  - Host: weights per-row results by 1/(n_c - 1), reduces in f64.

The device program is identical across cores (SPMD); all per-core
variation lives in the input data (permuted features + masks).
"""

import numpy as np
import ml_dtypes

TEMPERATURE = 0.1
INV_T = 1.0 / TEMPERATURE  # 10.0
EPS = 1e-12

B = 8192
D = 512
M = 8  # cores
R = B // M  # rows per core
KT = D // 128  # contraction tiles
NT = R // 128  # row tiles per core

_cache = {}


def _build(W, nw_chunks):
    """Build the SPMD Bass program for window width W (multiple of 512)."""
    from contextlib import ExitStack
    import concourse.bass as bass
    import concourse.tile as tile
    from concourse import bacc, mybir

    F32 = mybir.dt.float32
    BF16 = mybir.dt.bfloat16
    EXP = mybir.ActivationFunctionType.Exp
    LN = mybir.ActivationFunctionType.Ln
    ADD = mybir.AluOpType.add
    MUL = mybir.AluOpType.mult
    X = mybir.AxisListType.X

    nc = bacc.Bacc("TRN2", target_bir_lowering=False, debug=False)
    mov_d = nc.dram_tensor("mov", [D, B], BF16, kind="ExternalInput").ap()
    stat_d = nc.dram_tensor("stat", [D, R], BF16, kind="ExternalInput").ap()
    mcls_d = nc.dram_tensor("mcls", [R, W], BF16, kind="ExternalInput").ap()
    mpos_d = nc.dram_tensor("mpos", [R, W], BF16, kind="ExternalInput").ap()
    rowsum_d = nc.dram_tensor("rowsum", [R], F32, kind="ExternalOutput").ap()
    neg_d = nc.dram_tensor("neg_exp", [R], F32, kind="ExternalOutput").ap()

    nch = len(nw_chunks)

    with tile.TileContext(nc) as tc, ExitStack() as ctx:
        const = ctx.enter_context(tc.tile_pool(name="const", bufs=1))
        masks = ctx.enter_context(tc.tile_pool(name="masks", bufs=2))
        e1wp = ctx.enter_context(tc.tile_pool(name="e1wp", bufs=2))
        e1nwp = ctx.enter_context(tc.tile_pool(name="e1nwp", bufs=2))
        pw = ctx.enter_context(tc.tile_pool(name="pw", bufs=6))
        small = ctx.enter_context(tc.tile_pool(name="small", bufs=4))
        outp = ctx.enter_context(tc.tile_pool(name="outp", bufs=1))
        psw_pool = ctx.enter_context(tc.tile_pool(name="psw", bufs=1, space="PSUM"))
        psnw_pool = ctx.enter_context(tc.tile_pool(name="psnw", bufs=2, space="PSUM"))

        # Fine-grained input DMA: pieces in compute-consumption order
        # (non-window chunks first, window last), round-robined across
        # engine DMA queues so the PE can start within a few us.
        mov_sb = [
            const.tile([128, B], BF16, tag=f"mov{k}", name=f"mov{k}")
            for k in range(KT)
        ]
        stat_sb = []
        for k in range(KT):
            s = const.tile([128, R], BF16, tag=f"stat{k}")
            nc.gpsimd.dma_start(out=s, in_=stat_d[128 * k : 128 * (k + 1), :])
            stat_sb.append(s)
        dma_engines = [nc.sync, nc.scalar]
        pieces = [(W + sum(nw_chunks[:j]), c) for j, c in enumerate(nw_chunks)]
        pieces.append((0, W))
        qi = 0
        for col0, csz in pieces:
            for k in range(KT):
                dma_engines[qi % len(dma_engines)].dma_start(
                    out=mov_sb[k][:, col0 : col0 + csz],
                    in_=mov_d[128 * k : 128 * (k + 1), col0 : col0 + csz],
                )
                qi += 1

        rowsum_sb = outp.tile([128, NT], F32, tag="rowsum")
        neg_sb = outp.tile([128, NT], F32, tag="negdbg")
        eps_t = const.tile([128, 1], F32, tag="eps")
        nc.vector.memset(eps_t, EPS)

        for i in range(NT):
            mcls_t = masks.tile([128, W], BF16, tag="mcls")
            nc.gpsimd.dma_start(out=mcls_t, in_=mcls_d[128 * i : 128 * (i + 1), :])
            mpos_t = masks.tile([128, W], BF16, tag="mpos")
            nc.gpsimd.dma_start(out=mpos_t, in_=mpos_d[128 * i : 128 * (i + 1), :])

            strip = small.tile([128, 1 + nch], F32, tag="strip")
            stat_i = [stat_sb[k][:, 128 * i : 128 * (i + 1)] for k in range(KT)]

            # --- pass 1, non-window chunks ---
            col = W
            for j, csz in enumerate(nw_chunks):
                ps = psnw_pool.tile([128, csz], F32, tag="nw")
                for c0 in range(0, csz, 512):
                    for k in range(KT):
                        nc.tensor.matmul(
                            ps[:, c0 : c0 + 512],
                            stat_i[k],
                            mov_sb[k][:, col + c0 : col + c0 + 512],
                            start=(k == 0),
                            stop=(k == KT - 1),
                        )
                e1 = e1nwp.tile([128, csz], BF16, tag="e1nw")
                nc.scalar.activation(
                    out=e1, in_=ps, func=EXP, scale=INV_T,
                    accum_out=strip[:, 1 + j : 2 + j],
                )
                col += csz

            # --- pass 1, window chunk (kept in PSUM through pass 2) ---
            psw = psw_pool.tile([128, W], F32, tag="win")
            for c0 in range(0, W, 512):
                for k in range(KT):
                    nc.tensor.matmul(
                        psw[:, c0 : c0 + 512],
                        stat_i[k],
                        mov_sb[k][:, c0 : c0 + 512],
                        start=(k == 0),
                        stop=(k == KT - 1),
                    )
            e1w = e1wp.tile([128, W], F32, tag="e1w")
            nc.scalar.activation(
                out=e1w, in_=psw, func=EXP, scale=INV_T, accum_out=strip[:, 0:1]
            )

            # --- per-row scalars ---
            em = pw.tile([128, W], F32, tag="pw")
            cls = small.tile([128, 1], F32, tag="cls")
            nc.vector.tensor_tensor(out=em, in0=e1w, in1=mcls_t, op=MUL)
            nc.vector.tensor_reduce(out=cls, in_=em, axis=X, op=ADD)
            tot = small.tile([128, 1], F32, tag="tot")
            nc.vector.tensor_reduce(out=tot, in_=strip, axis=X, op=ADD)
            neg = small.tile([128, 1], F32, tag="neg")
            nc.vector.tensor_sub(neg, tot, cls)
            nc.vector.tensor_copy(out=neg_sb[:, i : i + 1], in_=neg)
            L = small.tile([128, 1], F32, tag="L")
            nc.scalar.activation(out=L, in_=neg, func=LN, bias=eps_t, scale=1.0)
            se = small.tile([128, 1], F32, tag="se")
            nc.scalar.activation(out=se, in_=L, func=EXP, scale=-1.0)

            # --- pass 2 (window only) ---
            lp = pw.tile([128, W], F32, tag="pw")
            nc.scalar.activation(out=lp, in_=em, func=LN, scale=se, bias=1.0)
            wt = pw.tile([128, W], F32, tag="pw")
            nc.scalar.activation(out=wt, in_=lp, func=EXP, scale=-2.0)
            t1 = pw.tile([128, W], F32, tag="pw")
            nc.vector.scalar_tensor_tensor(
                out=t1, in0=psw, scalar=-INV_T, in1=lp, op0=MUL, op1=ADD
            )
            t3 = pw.tile([128, W], F32, tag="pw")
            nc.vector.scalar_tensor_tensor(
                out=t3, in0=t1, scalar=L, in1=wt, op0=ADD, op1=MUL
            )
            junk = pw.tile([128, W], F32, tag="pw")
            nc.vector.tensor_tensor(out=junk, in0=t3, in1=mpos_t, op=MUL)
            nc.vector.tensor_reduce(
                out=rowsum_sb[:, i : i + 1], in_=junk, axis=X, op=ADD
            )

        nc.sync.dma_start(
            out=rowsum_d.rearrange("(t p) -> p t", p=128), in_=rowsum_sb
        )
        nc.sync.dma_start(out=neg_d.rearrange("(t p) -> p t", p=128), in_=neg_sb)

    # Exp and Ln live in different default ACT table sets; left alone the
    # table-load pass thrashes between them (~33 loads x 1.3us). Restrict
    # selection to the combined natural_log_exp_and_others set (positions
    # preserved so act_func_set_id still indexes act_info.json correctly).
    import concourse.bacc as bacc_mod

    orig_gat = bacc_mod.get_activation_tables

    def gat_combined(arch):
        t = orig_gat(arch)
        return {
            name: (funcs if name == "natural_log_exp_and_others" else set())
            for name, funcs in t.items()
        }

    bacc_mod.get_activation_tables = gat_combined
    try:
        nc.compile()
    finally:
        bacc_mod.get_activation_tables = orig_gat
    return nc


def _eval_class_order(perm_c, counts):
    """Max per-core column-union for a given class ordering."""
    csum = np.concatenate([[0], np.cumsum(counts[perm_c])])
    maxU = 0
    for r in range(M):
        lo_row, hi_row = r * R, (r + 1) * R
        first = int(np.searchsorted(csum, lo_row, side="right")) - 1
        last = int(np.searchsorted(csum, hi_row - 1, side="right")) - 1
        maxU = max(maxU, int(csum[last + 1] - csum[first]))
    return maxU


def _best_class_order(counts):
    """Choose a class ordering that minimizes the max per-core union."""
    ncls = len(counts)
    best = np.arange(ncls)
    bestU = _eval_class_order(best, counts)
    # heuristic: pair largest with smallest
    o = np.argsort(counts)[::-1]
    paired = np.empty(ncls, dtype=np.int64)
    paired[0::2] = o[: ncls // 2]
    paired[1::2] = o[ncls // 2 :][::-1]
    u = _eval_class_order(paired, counts)
    if u < bestU:
        best, bestU = paired, u
    rng = np.random.default_rng(0)
    cand = np.arange(ncls)
    for _ in range(4000):
        rng.shuffle(cand)
        u = _eval_class_order(cand, counts)
        if u < bestU:
            best, bestU = cand.copy(), u
            if bestU <= 1600:
                break
    return best, bestU


def _prep_inputs(features, labels):
    """Host-side sharding: per-core permutations, masks, class weights."""
    labels = np.asarray(labels).astype(np.int64)
    feats = np.asarray(features, dtype=np.float32)
    ncls = int(labels.max()) + 1
    counts = np.bincount(labels, minlength=ncls)
    class_order, maxU = _best_class_order(counts)
    # rank of each class in the chosen ordering
    rank = np.empty(ncls, dtype=np.int64)
    rank[class_order] = np.arange(ncls)
    order = np.argsort(rank[labels], kind="stable")
    sorted_ranks = rank[labels][order]
    bounds_by_rank = np.concatenate(
        [[0], np.cumsum(counts[class_order])]
    )

    cores = []
    for r in range(M):
        rows = order[r * R : (r + 1) * R]
        rks = sorted_ranks[r * R : (r + 1) * R]
        lo = int(bounds_by_rank[rks.min()])
        hi = int(bounds_by_rank[rks.max() + 1])
        cores.append((rows, lo, hi))

    W = int(-(-maxU // 512) * 512)
    if W > 3072:
        raise ValueError(f"class window {maxU} too large for PSUM budget")
    nw_total = B - W
    nwc = 1024 if W <= 2048 else 512
    nw_chunks = [nwc] * (nw_total // nwc)
    if nw_total % nwc:
        nw_chunks.append(nw_total % nwc)
    assert sum(nw_chunks) + W == B

    feats_bf = feats.astype(ml_dtypes.bfloat16)
    in_maps = []
    for r in range(M):
        rows, lo, hi = cores[r]
        win = order[lo:hi]
        rest = np.concatenate([order[:lo], order[hi:]])
        pad = W - len(win)
        # pad window with other-class cols (mask kills them)
        perm = np.concatenate([win, rest[:pad], rest[pad:]])
        wlab = labels[perm[:W]]
        rlab = labels[rows]
        eq = (wlab[None, :] == rlab[:, None])
        mcls = eq.astype(ml_dtypes.bfloat16)
        # positive mask: same class, excluding self
        self_col = np.full(R, -1, dtype=np.int64)
        colpos = {int(c): j for j, c in enumerate(perm[:W])}
        for p, g in enumerate(rows):
            self_col[p] = colpos[int(g)]
        mpos = eq.copy()
        mpos[np.arange(R), self_col] = False
        mpos = mpos.astype(ml_dtypes.bfloat16)
        in_maps.append(
            {
                "mov": np.ascontiguousarray(feats_bf[perm].T),
                "stat": np.ascontiguousarray(feats_bf[rows].T),
                "mcls": np.ascontiguousarray(mcls),
                "mpos": np.ascontiguousarray(mpos),
            }
        )
    return W, nw_chunks, in_maps, cores, counts


def _get_program(W, nw_chunks):
    key = (W, tuple(nw_chunks))
    if key not in _cache:
        _cache[key] = _build(W, nw_chunks)
    return _cache[key]


def _run(nc, in_maps, trace=False, trace_kwargs=None):
    from concourse.bass_utils import run_bass_kernel_spmd

    return run_bass_kernel_spmd(
        nc, in_maps, list(range(M)), trace=trace, trace_kwargs=trace_kwargs or {}
    )


def _finish(res_list, cores, counts, labels):
    labels = np.asarray(labels).astype(np.int64)
    w = np.zeros(16, dtype=np.float64)
    for c in range(16):
        n = counts[c]
        if n >= 2 and (B - n) > 0:
            w[c] = 1.0 / (n - 1)
    total = 0.0
    for r in range(M):
        rows, _, _ = cores[r]
        rs = np.asarray(res_list[r]["rowsum"], dtype=np.float64)
        total += float(np.dot(rs, w[labels[rows]]))
    return np.array(total / B, dtype=np.float32)


def kernel(features, labels):
    W, nw_chunks, in_maps, cores, counts = _prep_inputs(features, labels)
    nc = _get_program(W, nw_chunks)
    res = _run(nc, in_maps)
    return _finish(res.results, cores, counts, labels)


# revision 19
# speedup vs baseline: 1.4245x; 1.0345x over previous
"""Focal contrastive loss on 8 Trainium2 NeuronCores.

Strategy (data-parallel over rows, per-core column permutation):
  - Rows are sorted by label (16 classes). Core r owns 1024 consecutive
    sorted rows, which span 2-3 classes.  Each core receives its own
    column permutation of the full feature set in which the union of the
    classes touching its rows (the "window", <= W cols) comes first.
  - Device pass 1: sim block [1024, 8192] = statT.T @ movT via PE (bf16),
    exp(sim/T) row-sums via ACT accum (totals), masked class sums via DVE.
  - Device pass 2 (window cols only): focal terms
        t = -(1-pt)^2 * log(pt),  pt = sigmoid(z),  z = 10*sim - ln(d)
    computed as (LP - z) * exp(-2*LP) with LP = ln(1 + e^z); only Exp/Ln
    activation functions are used (one ACT table set).
  - Host: weights per-row results by 1/(n_c - 1), reduces in f64.

The device program is identical across cores (SPMD); all per-core
variation lives in the input data (permuted features + masks).
"""

import numpy as np
import ml_dtypes

TEMPERATURE = 0.1
INV_T = 1.0 / TEMPERATURE  # 10.0
EPS = 1e-12

B = 8192
D = 512
M = 8  # cores
R = B // M  # rows per core
KT = D // 128  # contraction tiles
NT = R // 128  # row tiles per core

_cache = {}


def _build(W, nw_chunks):
    """Build the SPMD Bass program for window width W (multiple of 512)."""
    from contextlib import ExitStack
    import concourse.bass as bass
    import concourse.tile as tile
    from concourse import bacc, mybir

    F32 = mybir.dt.float32
    BF16 = mybir.dt.bfloat16
    EXP = mybir.ActivationFunctionType.Exp
    LN = mybir.ActivationFunctionType.Ln
    ADD = mybir.AluOpType.add
    MUL = mybir.AluOpType.mult
    X = mybir.AxisListType.X

    nc = bacc.Bacc("TRN2", target_bir_lowering=False, debug=False)
    mov_d = nc.dram_tensor("mov", [D, B], BF16, kind="ExternalInput").ap()
    stat_d = nc.dram_tensor("stat", [D, R], BF16, kind="ExternalInput").ap()
    mcls_d = nc.dram_tensor("mcls", [R, W], BF16, kind="ExternalInput").ap()
    mpos_d = nc.dram_tensor("mpos", [R, W], BF16, kind="ExternalInput").ap()
    rowsum_d = nc.dram_tensor("rowsum", [R], F32, kind="ExternalOutput").ap()
    neg_d = nc.dram_tensor("neg_exp", [R], F32, kind="ExternalOutput").ap()

    nch = len(nw_chunks)

    with tile.TileContext(nc) as tc, ExitStack() as ctx:
        const = ctx.enter_context(tc.tile_pool(name="const", bufs=1))
        masks = ctx.enter_context(tc.tile_pool(name="masks", bufs=2))
        e1wp = ctx.enter_context(tc.tile_pool(name="e1wp", bufs=2))
        e1nwp = ctx.enter_context(tc.tile_pool(name="e1nwp", bufs=2))
        pw = ctx.enter_context(tc.tile_pool(name="pw", bufs=7))
        small = ctx.enter_context(tc.tile_pool(name="small", bufs=4))
        outp = ctx.enter_context(tc.tile_pool(name="outp", bufs=1))
        psw_pool = ctx.enter_context(tc.tile_pool(name="psw", bufs=1, space="PSUM"))
        psnw_pool = ctx.enter_context(tc.tile_pool(name="psnw", bufs=2, space="PSUM"))

        # Fine-grained input DMA: pieces in compute-consumption order
        # (non-window chunks first, window last), round-robined across
        # engine DMA queues so the PE can start within a few us.
        mov_sb = [
            const.tile([128, B], BF16, tag=f"mov{k}", name=f"mov{k}")
            for k in range(KT)
        ]
        stat_sb = []
        for k in range(KT):
            s = const.tile([128, R], BF16, tag=f"stat{k}")
            nc.gpsimd.dma_start(out=s, in_=stat_d[128 * k : 128 * (k + 1), :])
            stat_sb.append(s)
        dma_engines = [nc.sync, nc.scalar, nc.gpsimd]
        pieces = [(W + sum(nw_chunks[:j]), c) for j, c in enumerate(nw_chunks)]
        pieces.append((0, W))
        qi = 0
        for col0, csz in pieces:
            for k in range(KT):
                dma_engines[qi % len(dma_engines)].dma_start(
                    out=mov_sb[k][:, col0 : col0 + csz],
                    in_=mov_d[128 * k : 128 * (k + 1), col0 : col0 + csz],
                )
                qi += 1

        rowsum_sb = outp.tile([128, NT], F32, tag="rowsum")
        neg_sb = outp.tile([128, NT], F32, tag="negdbg")
        eps_t = const.tile([128, 1], F32, tag="eps")
        nc.vector.memset(eps_t, EPS)

        for i in range(NT):
            mcls_t = masks.tile([128, W], BF16, tag="mcls")
            nc.gpsimd.dma_start(out=mcls_t, in_=mcls_d[128 * i : 128 * (i + 1), :])
            mpos_t = masks.tile([128, W], BF16, tag="mpos")
            nc.gpsimd.dma_start(out=mpos_t, in_=mpos_d[128 * i : 128 * (i + 1), :])

            strip = small.tile([128, 1 + nch], F32, tag="strip")
            stat_i = [stat_sb[k][:, 128 * i : 128 * (i + 1)] for k in range(KT)]

            # --- pass 1, non-window chunks ---
            col = W
            for j, csz in enumerate(nw_chunks):
                ps = psnw_pool.tile([128, csz], F32, tag="nw")
                for c0 in range(0, csz, 512):
                    for k in range(KT):
                        nc.tensor.matmul(
                            ps[:, c0 : c0 + 512],
                            stat_i[k],
                            mov_sb[k][:, col + c0 : col + c0 + 512],
                            start=(k == 0),
                            stop=(k == KT - 1),
                        )
                e1 = e1nwp.tile([128, csz], BF16, tag="e1nw")
                nc.scalar.activation(
                    out=e1, in_=ps, func=EXP, scale=INV_T,
                    accum_out=strip[:, 1 + j : 2 + j],
                )
                col += csz

            # --- pass 1, window chunk (kept in PSUM through pass 2) ---
            psw = psw_pool.tile([128, W], F32, tag="win")
            for c0 in range(0, W, 512):
                for k in range(KT):
                    nc.tensor.matmul(
                        psw[:, c0 : c0 + 512],
                        stat_i[k],
                        mov_sb[k][:, c0 : c0 + 512],
                        start=(k == 0),
                        stop=(k == KT - 1),
                    )
            e1w = e1wp.tile([128, W], F32, tag="e1w")
            nc.scalar.activation(
                out=e1w, in_=psw, func=EXP, scale=INV_T, accum_out=strip[:, 0:1]
            )
            # copy sim window out of PSUM so the window banks free early
            simw = pw.tile([128, W], F32, tag="pw")
            nc.vector.tensor_copy(out=simw, in_=psw)

            # --- per-row scalars ---
            em = pw.tile([128, W], F32, tag="pw")
            cls = small.tile([128, 1], F32, tag="cls")
            nc.vector.scalar_tensor_tensor(
                out=em, in0=e1w, scalar=1.0, in1=mcls_t, op0=MUL, op1=MUL,
                accum_out=cls,
            )
            tot = small.tile([128, 1], F32, tag="tot")
            nc.vector.tensor_reduce(out=tot, in_=strip, axis=X, op=ADD)
            neg = small.tile([128, 1], F32, tag="neg")
            nc.vector.tensor_sub(neg, tot, cls)
            nc.vector.tensor_copy(out=neg_sb[:, i : i + 1], in_=neg)
            L = small.tile([128, 1], F32, tag="L")
            nc.scalar.activation(out=L, in_=neg, func=LN, bias=eps_t, scale=1.0)
            se = small.tile([128, 1], F32, tag="se")
            nc.scalar.activation(out=se, in_=L, func=EXP, scale=-1.0)

            # --- pass 2 (window only) ---
            lp = pw.tile([128, W], F32, tag="pw")
            nc.scalar.activation(out=lp, in_=em, func=LN, scale=se, bias=1.0)
            wt = pw.tile([128, W], F32, tag="pw")
            nc.scalar.activation(out=wt, in_=lp, func=EXP, scale=-2.0)
            t1 = pw.tile([128, W], F32, tag="pw")
            nc.vector.scalar_tensor_tensor(
                out=t1, in0=simw, scalar=-INV_T, in1=lp, op0=MUL, op1=ADD
            )
            t3 = pw.tile([128, W], F32, tag="pw")
            nc.vector.scalar_tensor_tensor(
                out=t3, in0=t1, scalar=L, in1=wt, op0=ADD, op1=MUL
            )
            junk = pw.tile([128, W], F32, tag="pw")
            nc.vector.scalar_tensor_tensor(
                out=junk, in0=t3, scalar=1.0, in1=mpos_t, op0=MUL, op1=MUL,
                accum_out=rowsum_sb[:, i : i + 1],
            )

        nc.sync.dma_start(
            out=rowsum_d.rearrange("(t p) -> p t", p=128), in_=rowsum_sb
        )
        nc.sync.dma_start(out=neg_d.rearrange("(t p) -> p t", p=128), in_=neg_sb)

    # Exp and Ln live in different default ACT table sets; left alone the
    # table-load pass thrashes between them (~33 loads x 1.3us). Restrict
    # selection to the combined natural_log_exp_and_others set (positions
    # preserved so act_func_set_id still indexes act_info.json correctly).
    import concourse.bacc as bacc_mod

    orig_gat = bacc_mod.get_activation_tables

    def gat_combined(arch):
        t = orig_gat(arch)
        return {
            name: (funcs if name == "natural_log_exp_and_others" else set())
            for name, funcs in t.items()
        }

    bacc_mod.get_activation_tables = gat_combined
    try:
        nc.compile()
    finally:
        bacc_mod.get_activation_tables = orig_gat
    return nc


def _eval_class_order(perm_c, counts):
    """Max per-core column-union for a given class ordering."""
    csum = np.concatenate([[0], np.cumsum(counts[perm_c])])
    maxU = 0
    for r in range(M):
        lo_row, hi_row = r * R, (r + 1) * R
        first = int(np.searchsorted(csum, lo_row, side="right")) - 1
        last = int(np.searchsorted(csum, hi_row - 1, side="right")) - 1
        maxU = max(maxU, int(csum[last + 1] - csum[first]))
    return maxU


def _best_class_order(counts):
    """Choose a class ordering that minimizes the max per-core union."""
    ncls = len(counts)
    best = np.arange(ncls)
    bestU = _eval_class_order(best, counts)
    # heuristic: pair largest with smallest
    o = np.argsort(counts)[::-1]
    paired = np.empty(ncls, dtype=np.int64)
    paired[0::2] = o[: ncls // 2]
    paired[1::2] = o[ncls // 2 :][::-1]
    u = _eval_class_order(paired, counts)
    if u < bestU:
        best, bestU = paired, u
    rng = np.random.default_rng(0)
    cand = np.arange(ncls)
    for _ in range(4000):
        rng.shuffle(cand)
        u = _eval_class_order(cand, counts)
        if u < bestU:
            best, bestU = cand.copy(), u
            if bestU <= 1600:
                break
    return best, bestU


def _prep_inputs(features, labels):
    """Host-side sharding: per-core permutations, masks, class weights."""
    labels = np.asarray(labels).astype(np.int64)
    feats = np.asarray(features, dtype=np.float32)
    ncls = int(labels.max()) + 1
    counts = np.bincount(labels, minlength=ncls)
    class_order, maxU = _best_class_order(counts)
    # rank of each class in the chosen ordering
    rank = np.empty(ncls, dtype=np.int64)
    rank[class_order] = np.arange(ncls)
    order = np.argsort(rank[labels], kind="stable")
    sorted_ranks = rank[labels][order]
    bounds_by_rank = np.concatenate(
        [[0], np.cumsum(counts[class_order])]
    )

    cores = []
    for r in range(M):
        rows = order[r * R : (r + 1) * R]
        rks = sorted_ranks[r * R : (r + 1) * R]
        lo = int(bounds_by_rank[rks.min()])
        hi = int(bounds_by_rank[rks.max() + 1])
        cores.append((rows, lo, hi))

    W = int(-(-maxU // 512) * 512)
    if W > 3072:
        raise ValueError(f"class window {maxU} too large for PSUM budget")
    nw_total = B - W
    nwc = 1024 if W <= 2048 else 512
    nw_chunks = [nwc] * (nw_total // nwc)
    if nw_total % nwc:
        nw_chunks.append(nw_total % nwc)
    assert sum(nw_chunks) + W == B

    feats_bf = feats.astype(ml_dtypes.bfloat16)
    in_maps = []
    for r in range(M):
        rows, lo, hi = cores[r]
        win = order[lo:hi]
        rest = np.concatenate([order[:lo], order[hi:]])
        pad = W - len(win)
        # pad window with other-class cols (mask kills them)
        perm = np.concatenate([win, rest[:pad], rest[pad:]])
        wlab = labels[perm[:W]]
        rlab = labels[rows]
        eq = (wlab[None, :] == rlab[:, None])
        mcls = eq.astype(ml_dtypes.bfloat16)
        # positive mask: same class, excluding self
        self_col = np.full(R, -1, dtype=np.int64)
        colpos = {int(c): j for j, c in enumerate(perm[:W])}
        for p, g in enumerate(rows):
            self_col[p] = colpos[int(g)]
        mpos = eq.copy()
        mpos[np.arange(R), self_col] = False
        mpos = mpos.astype(ml_dtypes.bfloat16)
        in_maps.append(
            {
                "mov": np.ascontiguousarray(feats_bf[perm].T),
                "stat": np.ascontiguousarray(feats_bf[rows].T),
                "mcls": np.ascontiguousarray(mcls),
                "mpos": np.ascontiguousarray(mpos),
            }
        )
    return W, nw_chunks, in_maps, cores, counts


def _get_program(W, nw_chunks):
    key = (W, tuple(nw_chunks))
    if key not in _cache:
        _cache[key] = _build(W, nw_chunks)
    return _cache[key]


def _run(nc, in_maps, trace=False, trace_kwargs=None):
    from concourse.bass_utils import run_bass_kernel_spmd

    return run_bass_kernel_spmd(
        nc, in_maps, list(range(M)), trace=trace, trace_kwargs=trace_kwargs or {}
    )


def _finish(res_list, cores, counts, labels):
    labels = np.asarray(labels).astype(np.int64)
    w = np.zeros(16, dtype=np.float64)
    for c in range(16):
        n = counts[c]
        if n >= 2 and (B - n) > 0:
            w[c] = 1.0 / (n - 1)
    total = 0.0
    for r in range(M):
        rows, _, _ = cores[r]
        rs = np.asarray(res_list[r]["rowsum"], dtype=np.float64)
        total += float(np.dot(rs, w[labels[rows]]))
    return np.array(total / B, dtype=np.float32)


def kernel(features, labels):
    W, nw_chunks, in_maps, cores, counts = _prep_inputs(features, labels)
    nc = _get_program(W, nw_chunks)
    res = _run(nc, in_maps)
    return _finish(res.results, cores, counts, labels)


# revision 27
# speedup vs baseline: 1.4776x; 1.0373x over previous
"""Focal contrastive loss on 8 Trainium2 NeuronCores.

Strategy (data-parallel over rows, per-core column permutation):
  - Rows are sorted by label (16 classes). Core r owns 1024 consecutive
    sorted rows, which span 2-3 classes.  Each core receives its own
    column permutation of the full feature set in which the union of the
    classes touching its rows (the "window", <= W cols) comes first.
  - Device pass 1: sim block [1024, 8192] = statT.T @ movT via PE (bf16),
    exp(sim/T) row-sums via ACT accum (totals), masked class sums via DVE.
  - Device pass 2 (window cols only): focal terms
        t = -(1-pt)^2 * log(pt),  pt = sigmoid(z),  z = 10*sim - ln(d)
    computed as (LP - z) * exp(-2*LP) with LP = ln(1 + e^z); only Exp/Ln
    activation functions are used (one ACT table set).
  - Host: weights per-row results by 1/(n_c - 1), reduces in f64.

The device program is identical across cores (SPMD); all per-core
variation lives in the input data (permuted features + masks).
"""

import numpy as np
import ml_dtypes

TEMPERATURE = 0.1
INV_T = 1.0 / TEMPERATURE  # 10.0
EPS = 1e-12

B = 8192
D = 512
M = 8  # cores
R = B // M  # rows per core
KT = D // 128  # contraction tiles
NT = R // 128  # row tiles per core

_cache = {}


def _build(W, nw_chunks):
    """Build the SPMD Bass program for window width W (multiple of 512)."""
    from contextlib import ExitStack
    import concourse.bass as bass
    import concourse.tile as tile
    from concourse import bacc, mybir

    F32 = mybir.dt.float32
    BF16 = mybir.dt.bfloat16
    FP8 = mybir.dt.float8e4
    EXP = mybir.ActivationFunctionType.Exp
    LN = mybir.ActivationFunctionType.Ln
    ADD = mybir.AluOpType.add
    MUL = mybir.AluOpType.mult
    X = mybir.AxisListType.X

    nc = bacc.Bacc("TRN2", target_bir_lowering=False, debug=False)
    mov_d = nc.dram_tensor("mov", [D, B], BF16, kind="ExternalInput").ap()
    stat_d = nc.dram_tensor("stat", [D, R], BF16, kind="ExternalInput").ap()
    mcls_d = nc.dram_tensor("mcls", [R, W], FP8, kind="ExternalInput").ap()
    mpos_d = nc.dram_tensor("mpos", [R, W], FP8, kind="ExternalInput").ap()
    rowsum_d = nc.dram_tensor("rowsum", [R], F32, kind="ExternalOutput").ap()
    neg_d = nc.dram_tensor("neg_exp", [R], F32, kind="ExternalOutput").ap()

    nch = len(nw_chunks)

    with tile.TileContext(nc) as tc, ExitStack() as ctx:
        const = ctx.enter_context(tc.tile_pool(name="const", bufs=1))
        masks = ctx.enter_context(tc.tile_pool(name="masks", bufs=2))
        e1wp = ctx.enter_context(tc.tile_pool(name="e1wp", bufs=2))
        e1nwp = ctx.enter_context(tc.tile_pool(name="e1nwp", bufs=2))
        pw = ctx.enter_context(tc.tile_pool(name="pw", bufs=7))
        small = ctx.enter_context(tc.tile_pool(name="small", bufs=4))
        outp = ctx.enter_context(tc.tile_pool(name="outp", bufs=1))
        psw_pool = ctx.enter_context(tc.tile_pool(name="psw", bufs=1, space="PSUM"))
        psnw_pool = ctx.enter_context(tc.tile_pool(name="psnw", bufs=2, space="PSUM"))

        # Fine-grained input DMA: pieces in compute-consumption order
        # (non-window chunks first, window last), round-robined across
        # engine DMA queues so the PE can start within a few us.
        mov_sb = [
            const.tile([128, B], BF16, tag=f"mov{k}", name=f"mov{k}")
            for k in range(KT)
        ]
        stat_sb = []
        for k in range(KT):
            s = const.tile([128, R], BF16, tag=f"stat{k}")
            nc.gpsimd.dma_start(out=s, in_=stat_d[128 * k : 128 * (k + 1), :])
            stat_sb.append(s)
        dma_engines = [nc.sync, nc.scalar, nc.gpsimd]
        # window pieces first: i_tile 0 computes its window block first
        pieces = [(0, W)]
        pieces += [(W + sum(nw_chunks[:j]), c) for j, c in enumerate(nw_chunks)]
        qi = 0
        for col0, csz in pieces:
            for k in range(KT):
                dma_engines[qi % len(dma_engines)].dma_start(
                    out=mov_sb[k][:, col0 : col0 + csz],
                    in_=mov_d[128 * k : 128 * (k + 1), col0 : col0 + csz],
                )
                qi += 1

        rowsum_sb = outp.tile([128, NT], F32, tag="rowsum")
        neg_sb = outp.tile([128, NT], F32, tag="negdbg")
        eps_t = const.tile([128, 1], F32, tag="eps")
        nc.vector.memset(eps_t, EPS)

        for i in range(NT):
            mcls_t = masks.tile([128, W], FP8, tag="mcls")
            nc.gpsimd.dma_start(out=mcls_t, in_=mcls_d[128 * i : 128 * (i + 1), :])
            mpos_t = masks.tile([128, W], FP8, tag="mpos")
            nc.gpsimd.dma_start(out=mpos_t, in_=mpos_d[128 * i : 128 * (i + 1), :])

            strip = small.tile([128, 1 + nch], F32, tag="strip")
            stat_i = [stat_sb[k][:, 128 * i : 128 * (i + 1)] for k in range(KT)]

            def do_window():
                psw = psw_pool.tile([128, W], F32, tag="win", name="psw")
                for c0 in range(0, W, 512):
                    for k in range(KT):
                        nc.tensor.matmul(
                            psw[:, c0 : c0 + 512],
                            stat_i[k],
                            mov_sb[k][:, c0 : c0 + 512],
                            start=(k == 0),
                            stop=(k == KT - 1),
                        )
                e1w = e1wp.tile([128, W], F32, tag="e1w", name="e1w")
                nc.scalar.activation(
                    out=e1w, in_=psw, func=EXP, scale=INV_T,
                    accum_out=strip[:, 0:1],
                )
                # copy sim window out of PSUM so the window banks free early
                simw = pw.tile([128, W], F32, tag="pw", name="simw")
                nc.vector.tensor_copy(out=simw, in_=psw)
                return e1w, simw

            def do_nw():
                col = W
                for j, csz in enumerate(nw_chunks):
                    ps = psnw_pool.tile([128, csz], F32, tag="nw", name="ps")
                    for c0 in range(0, csz, 512):
                        for k in range(KT):
                            nc.tensor.matmul(
                                ps[:, c0 : c0 + 512],
                                stat_i[k],
                                mov_sb[k][:, col + c0 : col + c0 + 512],
                                start=(k == 0),
                                stop=(k == KT - 1),
                            )
                    e1 = e1nwp.tile([128, csz], BF16, tag="e1nw", name="e1")
                    nc.scalar.activation(
                        out=e1, in_=ps, func=EXP, scale=INV_T,
                        accum_out=strip[:, 1 + j : 2 + j],
                    )
                    col += csz

            # i_tile 0 computes its window first (its cols arrive first);
            # later i_tiles do it last so the previous pass-2 can release
            # the window PSUM banks while their non-window chunks run.
            if i == 0:
                e1w, simw = do_window()
                do_nw()
            else:
                do_nw()
                e1w, simw = do_window()

            # --- per-row scalars ---
            em = pw.tile([128, W], F32, tag="pw")
            cls = small.tile([128, 1], F32, tag="cls")
            nc.vector.scalar_tensor_tensor(
                out=em, in0=e1w, scalar=1.0, in1=mcls_t, op0=MUL, op1=MUL,
                accum_out=cls,
            )
            tot = small.tile([128, 1], F32, tag="tot")
            nc.vector.tensor_reduce(out=tot, in_=strip, axis=X, op=ADD)
            neg = small.tile([128, 1], F32, tag="neg")
            nc.vector.tensor_sub(neg, tot, cls)
            nc.vector.tensor_copy(out=neg_sb[:, i : i + 1], in_=neg)
            L = small.tile([128, 1], F32, tag="L")
            nc.scalar.activation(out=L, in_=neg, func=LN, bias=eps_t, scale=1.0)
            se = small.tile([128, 1], F32, tag="se")
            nc.scalar.activation(out=se, in_=L, func=EXP, scale=-1.0)

            # --- pass 2 (window only) ---
            # Last i_tile: split into 512-col chunks so the ACT->DVE chain
            # pipelines (shortens the kernel tail); other i_tiles use full-W
            # ops (lower instruction overhead, latency hidden by next tile).
            csizes = (
                [512] * (W // 512) if i == NT - 1 else [W]
            )
            nparts = len(csizes)
            part = small.tile([128, nparts], F32, tag="part")
            c0 = 0
            for ci, cw in enumerate(csizes):
                sl = slice(c0, c0 + cw)
                lp = pw.tile([128, W], F32, tag="pw", name="lp")
                nc.scalar.activation(
                    out=lp[:, :cw], in_=em[:, sl], func=LN, scale=se, bias=1.0
                )
                wt = pw.tile([128, W], F32, tag="pw", name="wt")
                nc.scalar.activation(
                    out=wt[:, :cw], in_=lp[:, :cw], func=EXP, scale=-2.0
                )
                t1 = pw.tile([128, W], F32, tag="pw", name="t1")
                nc.vector.scalar_tensor_tensor(
                    out=t1[:, :cw], in0=simw[:, sl], scalar=-INV_T,
                    in1=lp[:, :cw], op0=MUL, op1=ADD,
                )
                t3 = pw.tile([128, W], F32, tag="pw", name="t3")
                nc.vector.scalar_tensor_tensor(
                    out=t3[:, :cw], in0=t1[:, :cw], scalar=L,
                    in1=wt[:, :cw], op0=ADD, op1=MUL,
                )
                junk = pw.tile([128, W], F32, tag="pw", name="junk")
                acc = (
                    rowsum_sb[:, i : i + 1]
                    if nparts == 1
                    else part[:, ci : ci + 1]
                )
                nc.vector.scalar_tensor_tensor(
                    out=junk[:, :cw], in0=t3[:, :cw], scalar=1.0,
                    in1=mpos_t[:, sl], op0=MUL, op1=MUL, accum_out=acc,
                )
                c0 += cw
            if nparts > 1:
                nc.vector.tensor_reduce(
                    out=rowsum_sb[:, i : i + 1], in_=part, axis=X, op=ADD
                )

        nc.sync.dma_start(
            out=rowsum_d.rearrange("(t p) -> p t", p=128), in_=rowsum_sb
        )
        nc.sync.dma_start(out=neg_d.rearrange("(t p) -> p t", p=128), in_=neg_sb)

    # Exp and Ln live in different default ACT table sets; left alone the
    # table-load pass thrashes between them (~33 loads x 1.3us). Restrict
    # selection to the combined natural_log_exp_and_others set (positions
    # preserved so act_func_set_id still indexes act_info.json correctly).
    import concourse.bacc as bacc_mod

    orig_gat = bacc_mod.get_activation_tables

    def gat_combined(arch):
        t = orig_gat(arch)
        return {
            name: (funcs if name == "natural_log_exp_and_others" else set())
            for name, funcs in t.items()
        }

    bacc_mod.get_activation_tables = gat_combined
    try:
        nc.compile()
    finally:
        bacc_mod.get_activation_tables = orig_gat
    return nc


def _eval_class_order(perm_c, counts):
    """Max per-core column-union for a given class ordering."""
    csum = np.concatenate([[0], np.cumsum(counts[perm_c])])
    maxU = 0
    for r in range(M):
        lo_row, hi_row = r * R, (r + 1) * R
        first = int(np.searchsorted(csum, lo_row, side="right")) - 1
        last = int(np.searchsorted(csum, hi_row - 1, side="right")) - 1
        maxU = max(maxU, int(csum[last + 1] - csum[first]))
    return maxU


def _best_class_order(counts):
    """Choose a class ordering that minimizes the max per-core union."""
    ncls = len(counts)
    best = np.arange(ncls)
    bestU = _eval_class_order(best, counts)
    # heuristic: pair largest with smallest
    o = np.argsort(counts)[::-1]
    paired = np.empty(ncls, dtype=np.int64)
    paired[0::2] = o[: ncls // 2]
    paired[1::2] = o[ncls // 2 :][::-1]
    u = _eval_class_order(paired, counts)
    if u < bestU:
        best, bestU = paired, u
    rng = np.random.default_rng(0)
    cand = np.arange(ncls)
    for _ in range(4000):
        rng.shuffle(cand)
        u = _eval_class_order(cand, counts)
        if u < bestU:
            best, bestU = cand.copy(), u
            if bestU <= 1600:
                break
    return best, bestU


def _prep_inputs(features, labels):
    """Host-side sharding: per-core permutations, masks, class weights."""
    labels = np.asarray(labels).astype(np.int64)
    feats = np.asarray(features, dtype=np.float32)
    ncls = int(labels.max()) + 1
    counts = np.bincount(labels, minlength=ncls)
    class_order, maxU = _best_class_order(counts)
    # rank of each class in the chosen ordering
    rank = np.empty(ncls, dtype=np.int64)
    rank[class_order] = np.arange(ncls)
    order = np.argsort(rank[labels], kind="stable")
    sorted_ranks = rank[labels][order]
    bounds_by_rank = np.concatenate(
        [[0], np.cumsum(counts[class_order])]
    )

    cores = []
    for r in range(M):
        rows = order[r * R : (r + 1) * R]
        rks = sorted_ranks[r * R : (r + 1) * R]
        lo = int(bounds_by_rank[rks.min()])
        hi = int(bounds_by_rank[rks.max() + 1])
        cores.append((rows, lo, hi))

    W = int(-(-maxU // 512) * 512)
    if W > 3072:
        raise ValueError(f"class window {maxU} too large for PSUM budget")
    nw_total = B - W
    nwc = 1024 if W <= 2048 else 512
    nw_chunks = [nwc] * (nw_total // nwc)
    if nw_total % nwc:
        nw_chunks.append(nw_total % nwc)
    assert sum(nw_chunks) + W == B

    feats_bf = feats.astype(ml_dtypes.bfloat16)
    in_maps = []
    for r in range(M):
        rows, lo, hi = cores[r]
        win = order[lo:hi]
        rest = np.concatenate([order[:lo], order[hi:]])
        pad = W - len(win)
        # pad window with other-class cols (mask kills them)
        perm = np.concatenate([win, rest[:pad], rest[pad:]])
        wlab = labels[perm[:W]]
        rlab = labels[rows]
        eq = (wlab[None, :] == rlab[:, None])
        mcls = eq.astype(ml_dtypes.float8_e4m3)
        # positive mask: same class, excluding self
        self_col = np.full(R, -1, dtype=np.int64)
        colpos = {int(c): j for j, c in enumerate(perm[:W])}
        for p, g in enumerate(rows):
            self_col[p] = colpos[int(g)]
        mpos = eq.copy()
        mpos[np.arange(R), self_col] = False
        mpos = mpos.astype(ml_dtypes.float8_e4m3)
        in_maps.append(
            {
                "mov": np.ascontiguousarray(feats_bf[perm].T),
                "stat": np.ascontiguousarray(feats_bf[rows].T),
                "mcls": np.ascontiguousarray(mcls),
                "mpos": np.ascontiguousarray(mpos),
            }
        )
    return W, nw_chunks, in_maps, cores, counts


def _get_program(W, nw_chunks):
    key = (W, tuple(nw_chunks))
    if key not in _cache:
        _cache[key] = _build(W, nw_chunks)
    return _cache[key]


def _run(nc, in_maps, trace=False, trace_kwargs=None):
    from concourse.bass_utils import run_bass_kernel_spmd

    return run_bass_kernel_spmd(
        nc, in_maps, list(range(M)), trace=trace, trace_kwargs=trace_kwargs or {}
    )


def _finish(res_list, cores, counts, labels):
    labels = np.asarray(labels).astype(np.int64)
    w = np.zeros(16, dtype=np.float64)
    for c in range(16):
        n = counts[c]
        if n >= 2 and (B - n) > 0:
            w[c] = 1.0 / (n - 1)
    total = 0.0
    for r in range(M):
        rows, _, _ = cores[r]
        rs = np.asarray(res_list[r]["rowsum"], dtype=np.float64)
        total += float(np.dot(rs, w[labels[rows]]))
    return np.array(total / B, dtype=np.float32)


def kernel(features, labels):
    W, nw_chunks, in_maps, cores, counts = _prep_inputs(features, labels)
    nc = _get_program(W, nw_chunks)
    res = _run(nc, in_maps)
    return _finish(res.results, cores, counts, labels)


# revision 28
# speedup vs baseline: 1.4917x; 1.0095x over previous
"""Focal contrastive loss on 8 Trainium2 NeuronCores.

Strategy (data-parallel over rows, per-core column permutation):
  - Rows are sorted by label (16 classes). Core r owns 1024 consecutive
    sorted rows, which span 2-3 classes.  Each core receives its own
    column permutation of the full feature set in which the union of the
    classes touching its rows (the "window", <= W cols) comes first.
  - Device pass 1: sim block [1024, 8192] = statT.T @ movT via PE (bf16),
    exp(sim/T) row-sums via ACT accum (totals), masked class sums via DVE.
  - Device pass 2 (window cols only): focal terms
        t = -(1-pt)^2 * log(pt),  pt = sigmoid(z),  z = 10*sim - ln(d)
    computed as (LP - z) * exp(-2*LP) with LP = ln(1 + e^z); only Exp/Ln
    activation functions are used (one ACT table set).
  - Host: weights per-row results by 1/(n_c - 1), reduces in f64.

The device program is identical across cores (SPMD); all per-core
variation lives in the input data (permuted features + masks).
"""

import numpy as np
import ml_dtypes

TEMPERATURE = 0.1
INV_T = 1.0 / TEMPERATURE  # 10.0
EPS = 1e-12

B = 8192
D = 512
M = 8  # cores
R = B // M  # rows per core
KT = D // 128  # contraction tiles
NT = R // 128  # row tiles per core

_cache = {}


def _build(W, nw_chunks):
    """Build the SPMD Bass program for window width W (multiple of 512)."""
    from contextlib import ExitStack
    import concourse.bass as bass
    import concourse.tile as tile
    from concourse import bacc, mybir

    F32 = mybir.dt.float32
    BF16 = mybir.dt.bfloat16
    FP8 = mybir.dt.float8e4
    EXP = mybir.ActivationFunctionType.Exp
    LN = mybir.ActivationFunctionType.Ln
    ADD = mybir.AluOpType.add
    MUL = mybir.AluOpType.mult
    X = mybir.AxisListType.X

    nc = bacc.Bacc("TRN2", target_bir_lowering=False, debug=False)
    mov_d = nc.dram_tensor("mov", [D, B], BF16, kind="ExternalInput").ap()
    stat_d = nc.dram_tensor("stat", [D, R], BF16, kind="ExternalInput").ap()
    mcls_d = nc.dram_tensor("mcls", [R, W], FP8, kind="ExternalInput").ap()
    mpos_d = nc.dram_tensor("mpos", [R, W], FP8, kind="ExternalInput").ap()
    rowsum_d = nc.dram_tensor("rowsum", [R], F32, kind="ExternalOutput").ap()
    neg_d = nc.dram_tensor("neg_exp", [R], F32, kind="ExternalOutput").ap()

    nch = len(nw_chunks)

    with tile.TileContext(nc) as tc, ExitStack() as ctx:
        const = ctx.enter_context(tc.tile_pool(name="const", bufs=1))
        masks = ctx.enter_context(tc.tile_pool(name="masks", bufs=2))
        e1wp = ctx.enter_context(tc.tile_pool(name="e1wp", bufs=2))
        e1nwp = ctx.enter_context(tc.tile_pool(name="e1nwp", bufs=2))
        pw = ctx.enter_context(tc.tile_pool(name="pw", bufs=7))
        small = ctx.enter_context(tc.tile_pool(name="small", bufs=4))
        outp = ctx.enter_context(tc.tile_pool(name="outp", bufs=1))
        psw_pool = ctx.enter_context(tc.tile_pool(name="psw", bufs=1, space="PSUM"))
        psnw_pool = ctx.enter_context(tc.tile_pool(name="psnw", bufs=2, space="PSUM"))

        # Fine-grained input DMA: pieces in compute-consumption order
        # (non-window chunks first, window last), round-robined across
        # engine DMA queues so the PE can start within a few us.
        mov_sb = [
            const.tile([128, B], BF16, tag=f"mov{k}", name=f"mov{k}")
            for k in range(KT)
        ]
        stat_sb = []
        for k in range(KT):
            s = const.tile([128, R], BF16, tag=f"stat{k}")
            nc.gpsimd.dma_start(out=s, in_=stat_d[128 * k : 128 * (k + 1), :])
            stat_sb.append(s)
        dma_engines = [nc.sync, nc.scalar, nc.gpsimd]
        # window pieces first: i_tile 0 computes its window block first
        pieces = [(0, W)]
        pieces += [(W + sum(nw_chunks[:j]), c) for j, c in enumerate(nw_chunks)]
        qi = 0
        for col0, csz in pieces:
            for k in range(KT):
                dma_engines[qi % len(dma_engines)].dma_start(
                    out=mov_sb[k][:, col0 : col0 + csz],
                    in_=mov_d[128 * k : 128 * (k + 1), col0 : col0 + csz],
                )
                qi += 1

        rowsum_sb = outp.tile([128, NT], F32, tag="rowsum")
        neg_sb = outp.tile([128, NT], F32, tag="negdbg")
        eps_t = const.tile([128, 1], F32, tag="eps")
        nc.vector.memset(eps_t, EPS)

        for i in range(NT):
            mcls_t = masks.tile([128, W], FP8, tag="mcls")
            nc.gpsimd.dma_start(out=mcls_t, in_=mcls_d[128 * i : 128 * (i + 1), :])
            mpos_t = masks.tile([128, W], FP8, tag="mpos")
            nc.gpsimd.dma_start(out=mpos_t, in_=mpos_d[128 * i : 128 * (i + 1), :])

            strip = small.tile([128, 1 + nch], F32, tag="strip")
            stat_i = [stat_sb[k][:, 128 * i : 128 * (i + 1)] for k in range(KT)]

            def do_window():
                psw = psw_pool.tile([128, W], F32, tag="win", name="psw")
                for c0 in range(0, W, 512):
                    for k in range(KT):
                        nc.tensor.matmul(
                            psw[:, c0 : c0 + 512],
                            stat_i[k],
                            mov_sb[k][:, c0 : c0 + 512],
                            start=(k == 0),
                            stop=(k == KT - 1),
                        )
                e1w = e1wp.tile([128, W], F32, tag="e1w", name="e1w")
                nc.scalar.activation(
                    out=e1w, in_=psw, func=EXP, scale=INV_T,
                    accum_out=strip[:, 0:1],
                )
                if i == NT - 1:
                    # no successor window: pass 2 can read PSUM directly
                    return e1w, psw
                # copy sim window out of PSUM so the window banks free early
                simw = pw.tile([128, W], F32, tag="pw", name="simw")
                nc.vector.tensor_copy(out=simw, in_=psw)
                return e1w, simw

            def do_nw():
                col = W
                for j, csz in enumerate(nw_chunks):
                    ps = psnw_pool.tile([128, csz], F32, tag="nw", name="ps")
                    for c0 in range(0, csz, 512):
                        for k in range(KT):
                            nc.tensor.matmul(
                                ps[:, c0 : c0 + 512],
                                stat_i[k],
                                mov_sb[k][:, col + c0 : col + c0 + 512],
                                start=(k == 0),
                                stop=(k == KT - 1),
                            )
                    e1 = e1nwp.tile([128, csz], BF16, tag="e1nw", name="e1")
                    nc.scalar.activation(
                        out=e1, in_=ps, func=EXP, scale=INV_T,
                        accum_out=strip[:, 1 + j : 2 + j],
                    )
                    col += csz

            # i_tile 0 computes its window first (its cols arrive first);
            # later i_tiles do it last so the previous pass-2 can release
            # the window PSUM banks while their non-window chunks run.
            if i == 0:
                e1w, simw = do_window()
                do_nw()
            else:
                do_nw()
                e1w, simw = do_window()

            # --- per-row scalars ---
            em = pw.tile([128, W], F32, tag="pw")
            cls = small.tile([128, 1], F32, tag="cls")
            nc.vector.scalar_tensor_tensor(
                out=em, in0=e1w, scalar=1.0, in1=mcls_t, op0=MUL, op1=MUL,
                accum_out=cls,
            )
            tot = small.tile([128, 1], F32, tag="tot")
            nc.vector.tensor_reduce(out=tot, in_=strip, axis=X, op=ADD)
            neg = small.tile([128, 1], F32, tag="neg")
            nc.vector.tensor_sub(neg, tot, cls)
            nc.vector.tensor_copy(out=neg_sb[:, i : i + 1], in_=neg)
            L = small.tile([128, 1], F32, tag="L")
            nc.scalar.activation(out=L, in_=neg, func=LN, bias=eps_t, scale=1.0)
            se = small.tile([128, 1], F32, tag="se")
            nc.scalar.activation(out=se, in_=L, func=EXP, scale=-1.0)

            # --- pass 2 (window only) ---
            # Last i_tile: split into 512-col chunks so the ACT->DVE chain
            # pipelines (shortens the kernel tail); other i_tiles use full-W
            # ops (lower instruction overhead, latency hidden by next tile).
            csizes = (
                [512] * (W // 512) if i == NT - 1 else [W]
            )
            nparts = len(csizes)
            part = small.tile([128, nparts], F32, tag="part")
            c0 = 0
            for ci, cw in enumerate(csizes):
                sl = slice(c0, c0 + cw)
                lp = pw.tile([128, W], F32, tag="pw", name="lp")
                nc.scalar.activation(
                    out=lp[:, :cw], in_=em[:, sl], func=LN, scale=se, bias=1.0
                )
                wt = pw.tile([128, W], F32, tag="pw", name="wt")
                nc.scalar.activation(
                    out=wt[:, :cw], in_=lp[:, :cw], func=EXP, scale=-2.0
                )
                t1 = pw.tile([128, W], F32, tag="pw", name="t1")
                nc.vector.scalar_tensor_tensor(
                    out=t1[:, :cw], in0=simw[:, sl], scalar=-INV_T,
                    in1=lp[:, :cw], op0=MUL, op1=ADD,
                )
                t3 = pw.tile([128, W], F32, tag="pw", name="t3")
                nc.vector.scalar_tensor_tensor(
                    out=t3[:, :cw], in0=t1[:, :cw], scalar=L,
                    in1=wt[:, :cw], op0=ADD, op1=MUL,
                )
                junk = pw.tile([128, W], F32, tag="pw", name="junk")
                acc = (
                    rowsum_sb[:, i : i + 1]
                    if nparts == 1
                    else part[:, ci : ci + 1]
                )
                nc.vector.scalar_tensor_tensor(
                    out=junk[:, :cw], in0=t3[:, :cw], scalar=1.0,
                    in1=mpos_t[:, sl], op0=MUL, op1=MUL, accum_out=acc,
                )
                c0 += cw
            if nparts > 1:
                nc.vector.tensor_reduce(
                    out=rowsum_sb[:, i : i + 1], in_=part, axis=X, op=ADD
                )

        nc.sync.dma_start(
            out=rowsum_d.rearrange("(t p) -> p t", p=128), in_=rowsum_sb
        )
        nc.sync.dma_start(out=neg_d.rearrange("(t p) -> p t", p=128), in_=neg_sb)

    # Exp and Ln live in different default ACT table sets; left alone the
    # table-load pass thrashes between them (~33 loads x 1.3us). Restrict
    # selection to the combined natural_log_exp_and_others set (positions
    # preserved so act_func_set_id still indexes act_info.json correctly).
    import concourse.bacc as bacc_mod

    orig_gat = bacc_mod.get_activation_tables

    def gat_combined(arch):
        t = orig_gat(arch)
        return {
            name: (funcs if name == "natural_log_exp_and_others" else set())
            for name, funcs in t.items()
        }

    bacc_mod.get_activation_tables = gat_combined
    try:
        nc.compile()
    finally:
        bacc_mod.get_activation_tables = orig_gat
    return nc


def _eval_class_order(perm_c, counts):
    """Max per-core column-union for a given class ordering."""
    csum = np.concatenate([[0], np.cumsum(counts[perm_c])])
    maxU = 0
    for r in range(M):
        lo_row, hi_row = r * R, (r + 1) * R
        first = int(np.searchsorted(csum, lo_row, side="right")) - 1
        last = int(np.searchsorted(csum, hi_row - 1, side="right")) - 1
        maxU = max(maxU, int(csum[last + 1] - csum[first]))
    return maxU


def _best_class_order(counts):
    """Choose a class ordering that minimizes the max per-core union."""
    ncls = len(counts)
    best = np.arange(ncls)
    bestU = _eval_class_order(best, counts)
    # heuristic: pair largest with smallest
    o = np.argsort(counts)[::-1]
    paired = np.empty(ncls, dtype=np.int64)
    paired[0::2] = o[: ncls // 2]
    paired[1::2] = o[ncls // 2 :][::-1]
    u = _eval_class_order(paired, counts)
    if u < bestU:
        best, bestU = paired, u
    rng = np.random.default_rng(0)
    cand = np.arange(ncls)
    for _ in range(4000):
        rng.shuffle(cand)
        u = _eval_class_order(cand, counts)
        if u < bestU:
            best, bestU = cand.copy(), u
            if bestU <= 1600:
                break
    return best, bestU


def _prep_inputs(features, labels):
    """Host-side sharding: per-core permutations, masks, class weights."""
    labels = np.asarray(labels).astype(np.int64)
    feats = np.asarray(features, dtype=np.float32)
    ncls = int(labels.max()) + 1
    counts = np.bincount(labels, minlength=ncls)
    class_order, maxU = _best_class_order(counts)
    # rank of each class in the chosen ordering
    rank = np.empty(ncls, dtype=np.int64)
    rank[class_order] = np.arange(ncls)
    order = np.argsort(rank[labels], kind="stable")
    sorted_ranks = rank[labels][order]
    bounds_by_rank = np.concatenate(
        [[0], np.cumsum(counts[class_order])]
    )

    cores = []
    for r in range(M):
        rows = order[r * R : (r + 1) * R]
        rks = sorted_ranks[r * R : (r + 1) * R]
        lo = int(bounds_by_rank[rks.min()])
        hi = int(bounds_by_rank[rks.max() + 1])
        cores.append((rows, lo, hi))

    W = int(-(-maxU // 512) * 512)
    if W > 3072:
        raise ValueError(f"class window {maxU} too large for PSUM budget")
    nw_total = B - W
    nwc = 1024 if W <= 2048 else 512
    nw_chunks = [nwc] * (nw_total // nwc)
    if nw_total % nwc:
        nw_chunks.append(nw_total % nwc)
    assert sum(nw_chunks) + W == B

    feats_bf = feats.astype(ml_dtypes.bfloat16)
    in_maps = []
    for r in range(M):
        rows, lo, hi = cores[r]
        win = order[lo:hi]
        rest = np.concatenate([order[:lo], order[hi:]])
        pad = W - len(win)
        # pad window with other-class cols (mask kills them)
        perm = np.concatenate([win, rest[:pad], rest[pad:]])
        wlab = labels[perm[:W]]
        rlab = labels[rows]
        eq = (wlab[None, :] == rlab[:, None])
        mcls = eq.astype(ml_dtypes.float8_e4m3)
        # positive mask: same class, excluding self
        self_col = np.full(R, -1, dtype=np.int64)
        colpos = {int(c): j for j, c in enumerate(perm[:W])}
        for p, g in enumerate(rows):
            self_col[p] = colpos[int(g)]
        mpos = eq.copy()
        mpos[np.arange(R), self_col] = False
        mpos = mpos.astype(ml_dtypes.float8_e4m3)
        in_maps.append(
            {
                "mov": np.ascontiguousarray(feats_bf[perm].T),
                "stat": np.ascontiguousarray(feats_bf[rows].T),
                "mcls": np.ascontiguousarray(mcls),
                "mpos": np.ascontiguousarray(mpos),
            }
        )
    return W, nw_chunks, in_maps, cores, counts


def _get_program(W, nw_chunks):
    key = (W, tuple(nw_chunks))
    if key not in _cache:
        _cache[key] = _build(W, nw_chunks)
    return _cache[key]


def _run(nc, in_maps, trace=False, trace_kwargs=None):
    from concourse.bass_utils import run_bass_kernel_spmd

    return run_bass_kernel_spmd(
        nc, in_maps, list(range(M)), trace=trace, trace_kwargs=trace_kwargs or {}
    )


def _finish(res_list, cores, counts, labels):
    labels = np.asarray(labels).astype(np.int64)
    w = np.zeros(16, dtype=np.float64)
    for c in range(16):
        n = counts[c]
        if n >= 2 and (B - n) > 0:
            w[c] = 1.0 / (n - 1)
    total = 0.0
    for r in range(M):
        rows, _, _ = cores[r]
        rs = np.asarray(res_list[r]["rowsum"], dtype=np.float64)
        total += float(np.dot(rs, w[labels[rows]]))
    return np.array(total / B, dtype=np.float32)


def kernel(features, labels):
    W, nw_chunks, in_maps, cores, counts = _prep_inputs(features, labels)
    nc = _get_program(W, nw_chunks)
    res = _run(nc, in_maps)
    return _finish(res.results, cores, counts, labels)
